# revision 9
# baseline (speedup 1.0000x reference)
"""Trainium2 Bass kernel for Ernie4.5-VL vision attention (ragged segments).

Contract: kernel(**inputs) takes the FULL unsharded inputs (keyed as in
setup_inputs()) and returns the FULL [S, D] float32 output.

Strategy (uniform 4x1024 segments, the shipped shape)
-----------------------------------------------------
8 cores = 2 head-groups x 4 segments. Per core: n_h=8 heads, 1024 tokens.
All matmuls run in bf16 (rel err ~5e-3 vs the 2e-2 gate); PSUM stays f32.

  phase V+QK (PE):
    v    = hidden @ Wv.T          token-major: out [tok, 640], written
                                  straight into the PV stationary layout
                                  (97-wide head slots, ones col at 96)
    qkT  = Wqk @ hidden.T         dim-major: 1280 packed q/k rows, zero pad
    RoPE runs on DVE/Pool as qk tiles complete (staged via DMA into
    0:40 / 64:104 row pairs, rotated in place).
  phase ATTN, per head (PSUM: 2 score slots + 2 po slots = 8 banks,
  score slots reuse the qk-phase psum tags):
    for each 128-key tile: ST = kT.T @ qT   (scores^T, [128,1024] psum)
                           PT = exp(ST)     (ACT, bf16 out, no max-sub)
                           po += vaug.T @ PT (accumulate, denom row at 96)
    normalize: den->recip->Pool broadcast->DVE mul (+v-bias), DMA into
    dense 5x128-row attn tiles.
  phase PROJ: out = Wproj_shard.T @ attn_dense   (dense K=640, 5 k-tiles)

Host does O(S*D) glue: packing, bf16 casts, summing the 2 per-token
group partials, bias add. Non-uniform cu_seqlens fall back to the legacy
8-way head-parallel f32r program (mode C).
"""

import os
import sys

import numpy as np

H = 16
HD = 80
BLK = 40  # rotate_half half-width
SCALE = HD ** -0.5
N_CORES = 8
D = 1280
NK = D // 128  # contraction tiles over the model dim
VW = 97  # vaug head-slot width: 80 dims + 16 zero + ones col at 96
KERNEL_DEBUG = bool(int(os.environ.get("KERNEL_DEBUG", "0")))


def _segments(cu_seqlens, S):
    """Intervals matching reference's searchsorted(cu[1:], i, 'right')."""
    b = np.clip(np.sort(np.asarray(cu_seqlens, dtype=np.int64)[1:5]), 0, S)
    bounds = [0] + list(b) + [S]
    segs = []
    for a, e in zip(bounds[:-1], bounds[1:]):
        if e > a:
            segs.append((int(a), int(e)))
    return segs


def _pieces(start, length, tile_rows=128):
    """Split global row range [start, start+length) into per-tile pieces."""
    out = []
    off = 0
    while off < length:
        g = start + off
        t, r = g // tile_rows, g % tile_rows
        n = min(tile_rows - r, length - off)
        out.append((t, r, n, off))
        off += n
    return out


# --------------------------------------------------------------------------
# uniform-mode program: 2 head-groups x 4 segments, bf16
# --------------------------------------------------------------------------

def _build_uniform(n_h, S_core):
    """Emit the SPMD program for one (head-group, segment) core.

    Engine-AP partition rules on TRN2 (walrus birverifier): compute-engine
    accesses must start at a 32-aligned partition and must not cross a
    64-boundary unless they start on one; cross-partition data movement
    must go through DMA. Layout choices follow from this (the 0:40/64:104
    rope row split, the denominator at vaug col 96).
    """
    import concourse.mybir as mybir
    import concourse.tile as tile
    from concourse import bacc
    from contextlib import ExitStack

    f32 = mybir.dt.float32
    bf16 = mybir.dt.bfloat16
    AF = mybir.ActivationFunctionType

    QK_ROWS = 4 * BLK * n_h          # 1280: q_lo|q_hi|k_lo|k_hi per head
    NJ = QK_ROWS // 128              # 10 qk output tiles (exact)
    VD = n_h * HD                    # 640 v dims
    NTT = S_core // 128              # 8 token tiles
    KP = (n_h * HD) // 128           # 5 dense proj k-tiles (exact)
    assert QK_ROWS % 128 == 0 and S_core % 128 == 0 and (n_h * HD) % 128 == 0

    nc = bacc.Bacc("TRN2", target_bir_lowering=False, debug=False,
                   enable_asserts=False, num_devices=N_CORES)

    hiddenT = nc.dram_tensor("hiddenT", [128, NK * S_core], bf16,
                             kind="ExternalInput").ap()
    wqkT = nc.dram_tensor("wqkT", [128, NK * QK_ROWS], bf16,
                          kind="ExternalInput").ap()
    wvT = nc.dram_tensor("wvT", [128, NK * VD], bf16,
                         kind="ExternalInput").ap()
    bias_qk2d = nc.dram_tensor("bias_qk2d", [128, NJ], f32,
                               kind="ExternalInput").ap()
    bias_v2d = nc.dram_tensor("bias_v2d", [128, n_h], f32,
                              kind="ExternalInput").ap()
    # cosP/sin2P [128, S]: rows 0:40 lo coeffs, 64:104 hi coeffs, rest 0.
    # sin2P lo rows hold -sin_lo, hi rows +sin_hi (rot = x*cos + swap(x)*s).
    cosP = nc.dram_tensor("cosP", [128, S_core], bf16,
                          kind="ExternalInput").ap()
    sin2P = nc.dram_tensor("sin2P", [128, S_core], bf16,
                           kind="ExternalInput").ap()
    wprojT = nc.dram_tensor("wprojT", [128, KP * D], bf16,
                            kind="ExternalInput").ap()
    # per (tt, head) vaug tail init: 16 zero cols + ones col at 96
    vinit = nc.dram_tensor("vinit", [128, NTT * n_h * (VW - HD)], bf16,
                           kind="ExternalInput").ap()
    outTb = nc.dram_tensor("outTb", [D, S_core], bf16,
                           kind="ExternalOutput").ap()
    if KERNEL_DEBUG:
        dbg_qk = nc.dram_tensor("dbg_qk", [128, NJ * S_core], bf16,
                                kind="ExternalOutput").ap()
        dbg_rot = nc.dram_tensor("dbg_rot", [128, 2 * n_h * S_core], bf16,
                                 kind="ExternalOutput").ap()
        dbg_vaug = nc.dram_tensor("dbg_vaug", [128, NTT * n_h * VW], bf16,
                                  kind="ExternalOutput").ap()
        dbg_ad = nc.dram_tensor("dbg_ad", [128, KP * S_core], bf16,
                                kind="ExternalOutput").ap()

    hid3 = hiddenT.rearrange("p (k s) -> p k s", k=NK)
    wqk3 = wqkT.rearrange("p (k m) -> p k m", k=NK)
    wv3 = wvT.rearrange("p (k m) -> p k m", k=NK)
    wp3 = wprojT.rearrange("p (k m) -> p k m", k=KP)

    with tile.TileContext(nc) as tc, ExitStack() as ctx:
        # ---------------- SBUF (everything fits; ~170KB/partition) --------
        # load order: hid+wv first (v phase starts earliest), then the
        # small persists, wqk (qk phase), vinit/zero-fills, wproj (late)
        hid_pool = ctx.enter_context(tc.tile_pool(name="hid", bufs=1))
        hid_sb = [hid_pool.tile([128, S_core], bf16, tag=f"hid{k}",
                                name=f"hid{k}") for k in range(NK)]
        w_pool = ctx.enter_context(tc.tile_pool(name="wts", bufs=1))
        wqk_sb = [w_pool.tile([128, QK_ROWS], bf16, tag=f"wqk{k}",
                              name=f"wqk{k}") for k in range(NK)]
        wv_sb = [w_pool.tile([128, VD], bf16, tag=f"wv{k}",
                             name=f"wv{k}") for k in range(NK)]
        wp_sb = [w_pool.tile([128, D], bf16, tag=f"wp{k}",
                             name=f"wp{k}") for k in range(KP)]
        for k in range(NK):
            nc.sync.dma_start(hid_sb[k][:], hid3[:, k, :])
            nc.sync.dma_start(wv_sb[k][:], wv3[:, k, :])

        persist = ctx.enter_context(tc.tile_pool(name="persist", bufs=1))
        cos_sb = persist.tile([128, S_core], bf16, tag="cos", name="cos")
        sin_sb = persist.tile([128, S_core], bf16, tag="sin", name="sin")
        bqk_sb = persist.tile([128, NJ], f32, tag="bqk", name="bqk")
        bv_sb = persist.tile([128, n_h], f32, tag="bv", name="bv")
        nc.sync.dma_start(cos_sb[:], cosP[:])
        nc.sync.dma_start(sin_sb[:], sin2P[:])
        nc.sync.dma_start(bqk_sb[:], bias_qk2d[:])
        nc.sync.dma_start(bv_sb[:], bias_v2d[:])
        for k in range(NK):
            nc.sync.dma_start(wqk_sb[k][:], wqk3[:, k, :])

        vaug_pool = ctx.enter_context(tc.tile_pool(name="vaug", bufs=1))
        vaug = vaug_pool.tile([128, NTT * n_h * VW], bf16, tag="vaug",
                              name="vaug")
        vaug4 = vaug.rearrange("p (t h c) -> p t h c", h=n_h, c=VW)
        vinit3 = vinit.rearrange("p (t h c) -> p t h c", h=n_h, c=VW - HD)
        nc.sync.dma_start(vaug4[:, :, :, HD:VW], vinit3[:, :, :, :])

        qk_pool = ctx.enter_context(tc.tile_pool(name="qkout", bufs=1))
        qk_sb = [qk_pool.tile([128, S_core], bf16, tag=f"qk{j}",
                              name=f"qk{j}") for j in range(NJ)]
        rot_pool = ctx.enter_context(tc.tile_pool(name="rot", bufs=1))
        stg = {}
        for h in range(n_h):
            for sec in ("q", "k"):
                stg[(sec, h)] = rot_pool.tile([128, S_core], bf16,
                                              tag=f"stg_{sec}{h}",
                                              name=f"stg_{sec}{h}")
        stgb_sb = [rot_pool.tile([128, S_core], bf16, tag=f"stgb{i}",
                                 name=f"stgb{i}") for i in range(2)]
        # zero rows 40:64 once (cos_sb rows 40:64 are host-zeroed); the
        # in-place rope multiplies junk there otherwise (NaN risk)
        for t in list(stg.values()) + stgb_sb:
            nc.sync.dma_start(t[BLK:64, :], cos_sb[BLK:64, :])
        for k in range(KP):
            nc.sync.dma_start(wp_sb[k][:], wp3[:, k, :])

        ad_pool = ctx.enter_context(tc.tile_pool(name="ad", bufs=1))
        ad_sb = [ad_pool.tile([128, S_core], bf16, tag=f"ad{k}",
                              name=f"ad{k}") for k in range(KP)]
        pt_pool = ctx.enter_context(tc.tile_pool(name="pt", bufs=3))
        ast_pool = ctx.enter_context(tc.tile_pool(name="ast", bufs=2))
        nrm_pool = ctx.enter_context(tc.tile_pool(name="nrm", bufs=2))
        ob_pool = ctx.enter_context(tc.tile_pool(name="ob", bufs=3))

        # ---------------- PSUM ------------------------------------------
        # big pool: 2x [128,1024] tags shared by qk-phase, attn scores and
        # proj (4 banks); v pool: 2x [128,512] (2 banks), closed before the
        # po pool (2x [128,1024], 4 banks) opens => peak 8 banks.
        ps_big = ctx.enter_context(tc.tile_pool(name="ps_big", bufs=1,
                                                space="PSUM"))
        ps_v_cm = tc.tile_pool(name="ps_v", bufs=1, space="PSUM")
        ps_v = ps_v_cm.__enter__()

        # ---------------- phase V: v = hidden @ Wv.T (token-major) -------
        ui = 0
        VC = 320  # v-dim chunk: 4 head slots per copy
        for tt in range(NTT):
            for c in range(VD // VC):
                pv = ps_v.tile([128, 512], f32, tag=f"pv{ui % 3}", name="pv")
                for k in range(NK):
                    nc.tensor.matmul(
                        pv[:, :VC],
                        hid_sb[k][:, tt * 128:(tt + 1) * 128],
                        wv_sb[k][:, c * VC:(c + 1) * VC],
                        start=(k == 0), stop=(k == NK - 1))
                h0 = c * (VC // HD)
                nc.scalar.activation(
                    vaug4[:, tt, h0:h0 + VC // HD, 0:HD],
                    pv[:, :VC].rearrange("p (h d) -> p h d", d=HD),
                    AF.Identity)
                ui += 1
        ps_v_cm.__exit__(None, None, None)

        # ---------------- phase QK + RoPE --------------------------------
        # head h occupies packed rows 160h..160h+160 (q_lo|q_hi|k_lo|k_hi);
        # rope for (sec,h) is emitted once the covering j tiles are done.
        # rot = A*cos + B*sin2 where A=[lo;hi], B=[hi;lo] at rows 0:40 /
        # 64:104 (SBUF tensor-tensor operands must share base partition,
        # so the swapped copy is staged via DMA, not read at offset).
        pair_box = [0]

        def emit_rope(sec, h):
            base = 4 * BLK * h + (0 if sec == "q" else 2 * BLK)
            tA = stg[(sec, h)]
            tB = stgb_sb[pair_box[0] % 2]
            pair_box[0] += 1
            # staging rides the Act queue: each DMA lands right behind the
            # qk-copy it depends on, so it never head-of-line blocks
            for dst, src in ((0, base), (64, base + BLK)):
                for (jt, r, n, off) in _pieces(src, BLK):
                    nc.scalar.dma_start(tA[dst + off:dst + off + n, :],
                                        qk_sb[jt][r:r + n, :])
            for dst, src in ((0, base + BLK), (64, base)):
                for (jt, r, n, off) in _pieces(src, BLK):
                    nc.scalar.dma_start(tB[dst + off:dst + off + n, :],
                                        qk_sb[jt][r:r + n, :])
            nc.vector.tensor_mul(tB[0:104, :], tB[0:104, :], sin_sb[0:104, :])
            nc.vector.tensor_mul(tA[0:104, :], tA[0:104, :], cos_sb[0:104, :])
            nc.vector.tensor_add(tA[0:104, :], tA[0:104, :], tB[0:104, :])

        rope_done = set()

        def emit_ready_ropes(rows_done):
            for h in range(n_h):
                for sec, hi in (("q", 4 * BLK * h + 2 * BLK),
                                ("k", 4 * BLK * h + 4 * BLK)):
                    if hi <= rows_done and (sec, h) not in rope_done:
                        rope_done.add((sec, h))
                        emit_rope(sec, h)

        for j in range(NJ):
            psj = ps_big.tile([128, S_core], f32, tag=f"t{j % 2}",
                              name="psqk")
            for k in range(NK):
                for half in range(S_core // 512):
                    nc.tensor.matmul(
                        psj[:, half * 512:(half + 1) * 512],
                        wqk_sb[k][:, j * 128:(j + 1) * 128],
                        hid_sb[k][:, half * 512:(half + 1) * 512],
                        start=(k == 0), stop=(k == NK - 1))
            nc.scalar.activation(qk_sb[j][:], psj[:], AF.Identity,
                                 bias=bqk_sb[:, j:j + 1])
            emit_ready_ropes((j + 1) * 128)

        if KERNEL_DEBUG:
            for j in range(NJ):
                nc.sync.dma_start(dbg_qk[:, j * S_core:(j + 1) * S_core],
                                  qk_sb[j][:])

        # ---------------- phase ATTN -------------------------------------
        ps_po = ctx.enter_context(tc.tile_pool(name="ps_po", bufs=1,
                                               space="PSUM"))
        def emit_scores(h, kt):
            ps = ps_big.tile([128, S_core], f32, tag=f"t{kt % 2}",
                             name="st")
            for half in range(S_core // 512):
                nc.tensor.matmul(
                    ps[:, half * 512:(half + 1) * 512],
                    stg[("k", h)][0:104, kt * 128:(kt + 1) * 128],
                    stg[("q", h)][0:104, half * 512:(half + 1) * 512],
                    start=True, stop=True)
            pt = pt_pool.tile([128, S_core], bf16, tag="pt", name="pt")
            nc.scalar.activation(pt[:], ps[:], AF.Exp)
            return pt

        for h in range(n_h):
            po = ps_po.tile([128, S_core], f32, tag=f"po{h % 2}", name="po")
            # one-deep software pipeline: scores run a tile ahead of PV so
            # the in-order PE never waits on the exp
            pts = [emit_scores(h, 0), emit_scores(h, 1)]
            for kt in range(NTT):
                pt = pts[kt]
                if kt + 2 < NTT:
                    pts.append(emit_scores(h, kt + 2))
                for half in range(S_core // 512):
                    nc.tensor.matmul(
                        po[:VW, half * 512:(half + 1) * 512],
                        vaug4[:, kt, h, :].rearrange("p c -> p c"),
                        pt[:, half * 512:(half + 1) * 512],
                        start=(kt == 0), stop=(kt == NTT - 1))
            # normalize per 512-half (shortens the po drain and the tail
            # into proj): denominator row 96 -> partition 0 (ucode reads
            # phys partition 0), reciprocal, broadcast, scale + v-bias
            rc = nrm_pool.tile([128, S_core], f32, tag="rc", name="rc")
            bc = nrm_pool.tile([128, S_core], f32, tag="bc", name="bc")
            ast = ast_pool.tile([128, S_core], bf16, tag="ast", name="ast")
            for c0 in range(0, S_core, 512):
                cs = slice(c0, c0 + 512)
                nc.gpsimd.tensor_copy(rc[96:97, cs], po[96:97, cs])
                nc.sync.dma_start(rc[0:1, cs], rc[96:97, cs])
                nc.vector.reciprocal(rc[0:1, cs], rc[0:1, cs])
                nc.gpsimd.partition_broadcast(bc[0:HD, cs], rc[0:1, cs])
                nc.vector.tensor_mul(ast[0:HD, cs], po[0:HD, cs],
                                     bc[0:HD, cs])
                nc.vector.tensor_scalar_add(ast[0:HD, cs], ast[0:HD, cs],
                                            bv_sb[0:HD, h:h + 1])
                # move into the dense proj-K layout (rows 80h.. global)
                for (kt_, r, n, off) in _pieces(HD * h, HD):
                    nc.sync.dma_start(ad_sb[kt_][r:r + n, cs],
                                      ast[off:off + n, cs])

        if KERNEL_DEBUG:
            i_ = 0
            for h in range(n_h):
                for sec in ("q", "k"):
                    nc.sync.dma_start(
                        dbg_rot[:, i_ * S_core:(i_ + 1) * S_core],
                        stg[(sec, h)][:])
                    i_ += 1
            nc.sync.dma_start(dbg_vaug[:], vaug[:])
            for k in range(KP):
                nc.sync.dma_start(dbg_ad[:, k * S_core:(k + 1) * S_core],
                                  ad_sb[k][:])

        # ---------------- phase PROJ -------------------------------------
        for j in range(D // 128):
            pj = ps_big.tile([128, S_core], f32, tag=f"t{j % 2}", name="pj")
            for k in range(KP):
                for half in range(S_core // 512):
                    nc.tensor.matmul(
                        pj[:, half * 512:(half + 1) * 512],
                        wp_sb[k][:, j * 128:(j + 1) * 128],
                        ad_sb[k][:, half * 512:(half + 1) * 512],
                        start=(k == 0), stop=(k == KP - 1))
            ob = ob_pool.tile([128, S_core], bf16, tag="ob", name="ob")
            nc.scalar.activation(ob[:], pj[:], AF.Identity)
            nc.sync.dma_start(outTb[j * 128:(j + 1) * 128, :], ob[:])

    nc.compile()
    return nc


def _tile_rows(x):
    """[R, C] with R = nk*128 -> [128, nk*C] k-major tiling."""
    R, C = x.shape
    nk = R // 128
    return np.ascontiguousarray(
        x.reshape(nk, 128, C).transpose(1, 0, 2).reshape(128, nk * C))


def _bf16(x):
    import ml_dtypes
    return np.ascontiguousarray(np.asarray(x, np.float32)).astype(
        ml_dtypes.bfloat16)


def _pack_uniform_group(Wqkv, bqkv, Wproj, heads):
    """Per-head-group weight fragments for the uniform-mode program."""
    n_h = len(heads)
    # packed qk rows: per head q_lo|q_hi|k_lo|k_hi, q pre-scaled
    Wqk = np.zeros((4 * BLK * n_h, D), np.float32)
    bqk = np.zeros((4 * BLK * n_h,), np.float32)
    for i, h in enumerate(heads):
        for sec_i, sec_off in enumerate((0, D)):  # q, k
            src = sec_off + h * HD
            w = Wqkv[src:src + HD, :]
            b = bqkv[src:src + HD]
            if sec_i == 0:
                w = w * SCALE
                b = b * SCALE
            r = 4 * BLK * i + sec_i * 2 * BLK
            Wqk[r:r + HD] = w
            bqk[r:r + HD] = b
    NJ = (4 * BLK * n_h) // 128
    wqkT = _tile_rows(_bf16(Wqk.T))
    bias_qk2d = np.ascontiguousarray(bqk.reshape(NJ, 128).T)
    # v weights, head-major [VD, D]
    Wv = np.zeros((n_h * HD, D), np.float32)
    bv = np.zeros((128, n_h), np.float32)
    for i, h in enumerate(heads):
        Wv[i * HD:(i + 1) * HD] = Wqkv[2 * D + h * HD:2 * D + (h + 1) * HD]
        bv[0:HD, i] = bqkv[2 * D + h * HD:2 * D + (h + 1) * HD]
    wvT = _tile_rows(_bf16(Wv.T))
    # proj rows for this group's dims, dense [n_h*HD, D]
    Wp = np.zeros((n_h * HD, D), np.float32)
    for i, h in enumerate(heads):
        Wp[i * HD:(i + 1) * HD] = Wproj[:, h * HD:(h + 1) * HD].T
    wprojT = _tile_rows(_bf16(Wp))
    return wqkT, bias_qk2d, wvT, bv, wprojT


def _pack_cos_sin(cos, sin):
    """cosP/sin2P [128, S] f32: lo coeffs rows 0:40, hi rows 64:104, rest 0.

    sin2P lo rows hold -sin_lo (multiply x_hi), hi rows +sin_hi (x_lo).
    """
    S = cos.shape[0]
    cosP = np.zeros((128, S), np.float32)
    sinP = np.zeros((128, S), np.float32)
    cosP[0:BLK] = cos.T[0:BLK]
    cosP[64:64 + BLK] = cos.T[BLK:HD]
    sinP[0:BLK] = -sin.T[0:BLK]
    sinP[64:64 + BLK] = sin.T[BLK:HD]
    return cosP, sinP


_CACHE = {}


def kernel(hidden_states, cos, sin, Wqkv, bqkv, Wproj, bproj, cu_seqlens):
    sys.path.insert(0, "/opt/trn_rl_repo")
    from concourse import bass_utils

    hidden_states = np.asarray(hidden_states, np.float32)
    cos = np.asarray(cos, np.float32)
    sin = np.asarray(sin, np.float32)
    Wqkv = np.asarray(Wqkv, np.float32)
    bqkv = np.asarray(bqkv, np.float32)
    Wproj = np.asarray(Wproj, np.float32)
    bproj = np.asarray(bproj, np.float32)

    S, D_ = hidden_states.shape
    assert D_ == D
    segs = _segments(cu_seqlens, S)
    uniform = (S % 4 == 0) and segs == [(i * S // 4, (i + 1) * S // 4)
                                        for i in range(4)] \
        and (S // 4) % 1024 == 0

    if uniform:
        n_h, S_core = H // 2, S // 4
        NTT = S_core // 128
        key = ("A2", S)
        if key not in _CACHE:
            _CACHE[key] = _build_uniform(n_h, S_core)
        nc = _CACHE[key]

        hiddenT = _bf16(hidden_states.T)
        cosP, sin2P = _pack_cos_sin(cos, sin)
        vinit = np.zeros((128, NTT * n_h * (VW - HD)), np.float32)
        vinit.reshape(128, NTT, n_h, VW - HD)[:, :, :, VW - HD - 1] = 1.0
        vinit = _bf16(vinit)

        group_frag = []
        for g in range(2):
            heads = list(range(g * n_h, (g + 1) * n_h))
            group_frag.append(_pack_uniform_group(Wqkv, bqkv, Wproj, heads))

        in_maps = []
        meta = []
        for g in range(2):
            wqkT, bias_qk2d, wvT, bv, wprojT = group_frag[g]
            for s in range(4):
                sl = slice(s * S_core, (s + 1) * S_core)
                in_maps.append({
                    "hiddenT": _tile_rows(hiddenT[:, sl]),
                    "wqkT": wqkT,
                    "wvT": wvT,
                    "bias_qk2d": bias_qk2d,
                    "bias_v2d": bv,
                    "cosP": _bf16(cosP[:, sl]),
                    "sin2P": _bf16(sin2P[:, sl]),
                    "wprojT": wprojT,
                    "vinit": vinit,
                })
                meta.append((g, s))
        res = bass_utils.run_bass_kernel_spmd(nc, in_maps,
                                              core_ids=list(range(N_CORES)))
        out = np.zeros((D, S), np.float32)
        for c, (g, s) in enumerate(meta):
            out[:, s * S_core:(s + 1) * S_core] += np.asarray(
                res.results[c]["outTb"], dtype=np.float32)
        if KERNEL_DEBUG:
            kernel._dbg = res.results
        return np.ascontiguousarray(out.T) + bproj[None, :]

    # ---------------- legacy fallback: 8-way head parallel, f32r ---------
    return _kernel_legacy(hidden_states, cos, sin, Wqkv, bqkv, Wproj,
                          bproj, cu_seqlens, segs)


# ==========================================================================
# legacy mode C (non-uniform cu_seqlens): 8-way head parallel, f32r
# ==========================================================================

ATTN_STRIDE = 96
MM_DT_NAME = "float32r"


def _pack_layout(n_h):
    ntiles = 2 * n_h
    pos = {}
    for h in range(n_h):
        for half in (0, 1):
            pos[("v", h, half)] = (2 * h + half, 0)
    qk = [("q", h, half) for h in range(n_h) for half in (0, 1)]
    qk += [("k", h, half) for h in range(n_h) for half in (0, 1)]
    for j, blk in enumerate(qk):
        pos[blk] = (j // 2, BLK + BLK * (j % 2))
    return pos, ntiles


def _build_program(n_h, S_core, segs_local):
    """Legacy streamed program (see kernel_baseline.py for commentary)."""
    import concourse.mybir as mybir
    import concourse.tile as tile
    from concourse import bacc
    from concourse.masks import make_identity
    from contextlib import ExitStack

    f32 = mybir.dt.float32
    mm_dt = getattr(mybir.dt, MM_DT_NAME)
    AF = mybir.ActivationFunctionType

    k_proj = n_h
    pos, n_mtiles = _pack_layout(n_h)
    dims_pad = n_mtiles * 128
    LVW = 97

    t_tiles = []
    for si, (a, e) in enumerate(segs_local):
        t = a
        while t < e:
            t_tiles.append((si, t, min(t + 128, e)))
            t += 128
    n_tt = len(t_tiles)

    nc = bacc.Bacc("TRN2", target_bir_lowering=False, debug=False,
                   enable_asserts=False, num_devices=N_CORES)

    hiddenT = nc.dram_tensor("hiddenT", [128, NK * S_core], mm_dt,
                             kind="ExternalInput").ap()
    wqkvT = nc.dram_tensor("wqkvT", [128, NK * dims_pad], mm_dt,
                           kind="ExternalInput").ap()
    bias2d = nc.dram_tensor("bias2d", [128, n_mtiles], f32,
                            kind="ExternalInput").ap()
    cosP = nc.dram_tensor("cosP", [128, S_core], mm_dt,
                          kind="ExternalInput").ap()
    sin2P = nc.dram_tensor("sin2P", [128, S_core], mm_dt,
                           kind="ExternalInput").ap()
    wprojT = nc.dram_tensor("wprojT", [n_h * HD, D], mm_dt,
                            kind="ExternalInput").ap()
    vinit = nc.dram_tensor("vinit", [128, n_tt * (LVW - HD)], mm_dt,
                           kind="ExternalInput").ap()
    outT = nc.dram_tensor("outT", [D, S_core], f32, kind="ExternalOutput").ap()

    def r_(ap):
        return ap.bitcast(mm_dt)

    BC = 1024
    big_chunks = [(c, min(c + BC, S_core)) for c in range(0, S_core, BC)]

    def halves(c0, c1):
        out = []
        q = c0
        while q < c1:
            out.append((q, min(q + 512, c1)))
            q = q + 512
        return out

    with tile.TileContext(nc) as tc, ExitStack() as ctx:
        persist = ctx.enter_context(tc.tile_pool(name="persist", bufs=1))
        ident = persist.tile([128, 128], f32, tag="ident", name="ident")
        make_identity(nc, ident[:])
        bias_sb = persist.tile([128, n_mtiles], f32, tag="bias", name="bias")
        nc.sync.dma_start(bias_sb[:], bias2d[:])

        psum_all_cm = tc.tile_pool(name="psum_all", bufs=1, space="PSUM")
        psum_all = psum_all_cm.__enter__()
        qkv_pool = ctx.enter_context(tc.tile_pool(name="big", bufs=1))
        qkv_sb = [qkv_pool.tile([128, S_core], mm_dt, tag=f"qkvT{j}",
                                name=f"qkvT{j}") for j in range(n_mtiles)]
        rot_cm = tc.tile_pool(name="rot", bufs=1)
        rv = rot_cm.__enter__()
        rot_sb = {}
        for h in range(n_h):
            for sec in ("q", "k"):
                rot_sb[(sec, h)] = rv.tile([128, S_core], mm_dt,
                                           tag=f"rot_{sec}{h}",
                                           name=f"rot_{sec}{h}")
        RC = 1024
        rope_cm = tc.tile_pool(name="rope_scr", bufs=2)
        rope_scr = rope_cm.__enter__()

        hidden3 = hiddenT.rearrange("p (k s) -> p k s", k=NK)
        w3 = wqkvT.rearrange("p (k m) -> p k m", k=NK)
        with ExitStack() as p1:
            w_pool = p1.enter_context(tc.tile_pool(name="wres", bufs=1))
            w_sb = [w_pool.tile([128, dims_pad], mm_dt, tag=f"w{k}",
                                name=f"w{k}") for k in range(NK)]
            for k in range(NK):
                nc.sync.dma_start(w_sb[k][:], w3[:, k, :])
            assert n_mtiles == 4
            hid_pool = p1.enter_context(tc.tile_pool(name="hidstream",
                                                     bufs=3))
            for (h0, h1) in halves(0, S_core):
                hw = h1 - h0
                ps01 = psum_all.tile([128, BC], f32, tag="t0", name="ps01")
                ps23 = psum_all.tile([128, BC], f32, tag="t1", name="ps23")
                pj_of = lambda j: (ps01 if j < 2 else ps23, (j % 2) * 512)
                for k in range(NK):
                    ht = hid_pool.tile([128, 512], mm_dt, tag="hidc",
                                       name="hidc")
                    nc.sync.dma_start(ht[:, :hw], hidden3[:, k, h0:h1])
                    for j in range(n_mtiles):
                        psj, co = pj_of(j)
                        nc.tensor.matmul(
                            psj[:, co:co + hw],
                            r_(w_sb[k][:, j * 128:(j + 1) * 128]),
                            r_(ht[:, :hw]),
                            start=(k == 0), stop=(k == NK - 1))
                for j in range(n_mtiles):
                    psj, co = pj_of(j)
                    nc.scalar.activation(qkv_sb[j][:, h0:h1],
                                         psj[:, co:co + hw], AF.Identity,
                                         bias=bias_sb[:, j:j + 1])

        psum_all_cm.__exit__(None, None, None)
        ps_att = ctx.enter_context(tc.tile_pool(name="ps_att", bufs=1,
                                                space="PSUM"))

        stg = {}
        for nm in ("sa0", "sa1", "sb0", "sb1"):
            stg[nm] = rope_scr.tile([128, RC], mm_dt, tag=nm, name=nm, bufs=1)
        pair_i = 0
        for ci, f0 in enumerate(range(0, S_core, RC)):
            f1 = min(f0 + RC, S_core)
            fs = f1 - f0
            cos_sb = rope_scr.tile([128, RC], mm_dt, tag="cos", name="cos",
                                   bufs=1)
            sin_sb = rope_scr.tile([128, RC], mm_dt, tag="sin", name="sin",
                                   bufs=1)
            nc.scalar.dma_start(cos_sb[:, :fs], cosP[:, f0:f1])
            nc.scalar.dma_start(sin_sb[:, :fs], sin2P[:, f0:f1])
            if ci == 0:
                for nm in stg:
                    nc.scalar.dma_start(stg[nm][BLK:64, :], cos_sb[BLK:64, :])
            for h in range(n_h):
                for sec in ("q", "k"):
                    lo_t, lo_r = pos[(sec, h, 0)]
                    hi_t, hi_r = pos[(sec, h, 1)]
                    assert hi_t == lo_t and hi_r == lo_r + BLK
                    x = qkv_sb[lo_t]
                    dst = rot_sb[(sec, h)]
                    stga = stg[f"sa{pair_i % 2}"]
                    stgb = stg[f"sb{pair_i % 2}"]
                    nc.scalar.dma_start(stga[0:BLK, :fs],
                                        x[lo_r:lo_r + BLK, f0:f1])
                    nc.scalar.dma_start(stga[64:64 + BLK, :fs],
                                        x[hi_r:hi_r + BLK, f0:f1])
                    nc.scalar.dma_start(stgb[0:BLK, :fs],
                                        x[hi_r:hi_r + BLK, f0:f1])
                    nc.scalar.dma_start(stgb[64:64 + BLK, :fs],
                                        x[lo_r:lo_r + BLK, f0:f1])
                    nc.vector.tensor_mul(dst[0:104, f0:f1], stga[0:104, :fs],
                                         cos_sb[0:104, :fs])
                    eng = nc.gpsimd if pair_i % 2 == 0 else nc.vector
                    eng.tensor_mul(stgb[0:104, :fs], stgb[0:104, :fs],
                                   sin_sb[0:104, :fs])
                    nc.vector.tensor_add(dst[0:104, f0:f1], dst[0:104, f0:f1],
                                         stgb[0:104, :fs])
                    pair_i += 1
        rope_cm.__exit__(None, None, None)

        vaug_cm = tc.tile_pool(name="vaug", bufs=1)
        vaug_pool = vaug_cm.__enter__()
        vaug_sb = [vaug_pool.tile([128, n_tt * LVW], mm_dt, tag=f"vaug{h}",
                                  name=f"vaug{h}") for h in range(n_h)]
        vinit3 = vinit.rearrange("p (t c) -> p t c", c=LVW - HD)
        for h in range(n_h):
            nc.sync.dma_start(
                vaug_sb[h].rearrange("p (t c) -> p t c", c=LVW)[:, :, HD:LVW],
                vinit3[:, :, :])
        GRP = 4

        def emit_vaug(h):
            gi = 0
            while gi < n_tt:
                hi_g = min(gi + GRP, n_tt)
                if all(t_tiles[g][2] - t_tiles[g][1] == 128
                       for g in range(gi, hi_g)):
                    grp = list(range(gi, hi_g))
                else:
                    grp = [gi]
                ng = len(grp)
                tp = ps_att.tile([128, GRP * HD], f32, tag="tp", name="tp")
                for x, g in enumerate(grp):
                    si, t0, t1 = t_tiles[g]
                    sz = t1 - t0
                    for half in (0, 1):
                        vt, vr = pos[("v", h, half)]
                        nc.tensor.transpose(
                            tp[:sz, x * HD + half * BLK:
                               x * HD + (half + 1) * BLK],
                            qkv_sb[vt][0:BLK, t0:t1].bitcast(f32),
                            ident[:BLK, :BLK])
                sz0 = t_tiles[grp[0]][2] - t_tiles[grp[0]][1]
                dst = vaug_sb[h].rearrange("p (t c) -> p t c", c=LVW)
                src_ap = tp.rearrange("p (t c) -> p t c", c=HD)
                if h % 2 == 0:
                    nc.vector.tensor_copy(dst[:sz0, grp[0]:grp[0] + ng, 0:HD],
                                          src_ap[:sz0, 0:ng, :])
                else:
                    nc.scalar.activation(dst[:sz0, grp[0]:grp[0] + ng, 0:HD],
                                         src_ap[:sz0, 0:ng, :], AF.Identity)
                gi += ng

        attn_sb = [qkv_pool.tile([128, S_core], mm_dt, tag=f"qkvT{h}",
                                 name=f"attnT{h}") for h in range(n_h)]

        seg_ttiles = {}
        for ti, (si, t0, t1) in enumerate(t_tiles):
            seg_ttiles.setdefault(si, []).append((ti, t0, t1))

        BA = 512
        with ExitStack() as p4:
            pt_pool = p4.enter_context(tc.tile_pool(name="pt", bufs=3))
            nrm_pool = p4.enter_context(tc.tile_pool(name="nrm", bufs=2))
            unit_box = [0]

            def emit_attention(h, si, a, e):
                qT = rot_sb[("q", h)]
                kT = rot_sb[("k", h)]
                q = a
                while q < e:
                    q0, q1 = q, min(q + BA, e)
                    qs = q1 - q0
                    po = ps_att.tile([128, BA], f32,
                                     tag=f"po{unit_box[0] % 2}", name="pv")
                    tts = seg_ttiles[si]
                    for idx, (ti, t0, t1) in enumerate(tts):
                        sz = t1 - t0
                        ps = ps_att.tile([128, BA], f32, tag=f"st{idx % 2}",
                                         name="st")
                        nc.tensor.matmul(ps[:sz, :qs], r_(kT[0:104, t0:t1]),
                                         r_(qT[0:104, q0:q1]),
                                         start=True, stop=True)
                        pt = pt_pool.tile([128, BA], mm_dt, tag="pt", name="pt")
                        nc.scalar.activation(pt[:sz, :qs], ps[:sz, :qs], AF.Exp)
                        nc.tensor.matmul(
                            po[:LVW, :qs],
                            r_(vaug_sb[h][:sz, ti * LVW:(ti + 1) * LVW]),
                            r_(pt[:sz, :qs]),
                            start=(idx == 0), stop=(idx == len(tts) - 1))
                    rc = nrm_pool.tile([128, BA], f32, tag="rc", name="rc")
                    nc.vector.tensor_copy(rc[96:97, :qs], po[96:97, :qs])
                    nc.sync.dma_start(rc[0:1, :qs], rc[96:97, :qs])
                    nc.vector.reciprocal(rc[0:1, :qs], rc[0:1, :qs])
                    bc = nrm_pool.tile([128, BA], mm_dt, tag="bc", name="bc")
                    nc.gpsimd.partition_broadcast(
                        bc[0:HD, :qs], rc[0:1, :qs].bitcast(mm_dt))
                    nc.vector.tensor_mul(attn_sb[h][0:HD, q0:q1],
                                         po[0:HD, :qs], bc[0:HD, :qs])
                    unit_box[0] += 1
                    q = q1

            for h in range(n_h):
                emit_vaug(h)
            for si, (a, e) in enumerate(segs_local):
                for h in range(n_h):
                    emit_attention(h, si, a, e)

        vaug_cm.__exit__(None, None, None)
        rot_cm.__exit__(None, None, None)

        with ExitStack() as p5:
            wp_pool = p5.enter_context(tc.tile_pool(name="wp", bufs=1))
            wp_sb = []
            for kt in range(k_proj):
                t = wp_pool.tile([HD, D], mm_dt, tag=f"wp{kt}", name=f"wp{kt}")
                nc.sync.dma_start(t[:], wprojT[kt * HD:(kt + 1) * HD, :])
                wp_sb.append(t)
            out_pool = p5.enter_context(tc.tile_pool(name="outsb", bufs=3))
            for (c0, c1) in big_chunks:
                cs = c1 - c0
                for j in range(D // 128):
                    ob = out_pool.tile([128, BC], f32, tag="ob", name="ob")
                    for (h0, h1) in halves(c0, c1):
                        ps = ps_att.tile([128, 512], f32, tag=f"st{j % 2}",
                                         name="pj")
                        for kt in range(k_proj):
                            nc.tensor.matmul(
                                ps[:, :h1 - h0],
                                r_(wp_sb[kt][:, j * 128:(j + 1) * 128]),
                                r_(attn_sb[kt][0:HD, h0:h1]),
                                start=(kt == 0), stop=(kt == k_proj - 1))
                        if j % 2 == 0:
                            nc.vector.tensor_copy(ob[:, h0 - c0:h1 - c0],
                                                  ps[:, :h1 - h0])
                        else:
                            nc.scalar.activation(ob[:, h0 - c0:h1 - c0],
                                                 ps[:, :h1 - h0], AF.Identity)
                    nc.sync.dma_start(outT[j * 128:(j + 1) * 128, c0:c1],
                                      ob[:, :cs])

    nc.compile()
    return nc


def _pack_w_legacy(Wqkv, bqkv, heads, n_h):
    pos, n_mtiles = _pack_layout(n_h)
    dims_pad = n_mtiles * 128
    W = np.zeros((dims_pad, D), np.float32)
    b = np.zeros((dims_pad,), np.float32)
    sec_off = {"q": 0, "k": D, "v": 2 * D}
    for i, h in enumerate(heads):
        for sec in ("q", "k", "v"):
            for half in (0, 1):
                t, r = pos[(sec, i, half)]
                src = sec_off[sec] + h * HD + half * BLK
                w = Wqkv[src:src + BLK, :]
                bb = bqkv[src:src + BLK]
                if sec == "q":
                    w = w * SCALE
                    bb = bb * SCALE
                W[t * 128 + r:t * 128 + r + BLK] = w
                b[t * 128 + r:t * 128 + r + BLK] = bb
    w_tiled = _tile_rows(np.ascontiguousarray(W.T))
    bias2d = np.ascontiguousarray(b.reshape(n_mtiles, 128).T)
    return w_tiled, bias2d


def _pack_wproj_legacy(Wproj, heads):
    W = np.zeros((len(heads) * HD, Wproj.shape[0]), np.float32)
    for i, h in enumerate(heads):
        W[i * HD:(i + 1) * HD] = Wproj[:, h * HD:(h + 1) * HD].T
    return W


def _kernel_legacy(hidden_states, cos, sin, Wqkv, bqkv, Wproj, bproj,
                   cu_seqlens, segs):
    from concourse import bass_utils

    S = hidden_states.shape[0]
    hiddenT = np.ascontiguousarray(hidden_states.T)
    cosP, sin2P = _pack_cos_sin(cos, sin)

    n_h, S_core = H // N_CORES, S
    key = ("C", S, tuple(np.asarray(cu_seqlens).tolist()))
    if key not in _CACHE:
        _CACHE[key] = _build_program(n_h, S_core, segs)
    nc = _CACHE[key]
    n_tt = sum(-(-(e - a) // 128) for a, e in segs)
    vinit = np.zeros((128, n_tt, 17), np.float32)
    vinit[:, :, 16] = 1.0
    vinit = np.ascontiguousarray(vinit.reshape(128, n_tt * 17))
    hid_tiled = _tile_rows(hiddenT)
    in_maps = []
    for c in range(N_CORES):
        heads = list(range(c * n_h, (c + 1) * n_h))
        wt, b2 = _pack_w_legacy(Wqkv, bqkv, heads, n_h)
        in_maps.append({
            "hiddenT": hid_tiled,
            "wqkvT": wt,
            "bias2d": b2,
            "cosP": cosP,
            "sin2P": sin2P,
            "wprojT": _pack_wproj_legacy(Wproj, heads),
            "vinit": vinit,
        })
    res = bass_utils.run_bass_kernel_spmd(nc, in_maps,
                                          core_ids=list(range(N_CORES)))
    out = np.zeros((D, S), np.float32)
    for c in range(N_CORES):
        out += res.results[c]["outT"]
    return np.ascontiguousarray(out.T) + bproj[None, :]
